# revision 26
# baseline (speedup 1.0000x reference)
"""Trainium2 Bass kernel for nn_CirculantSTRING (v6).

Math: out[b,n,:] = irfft(exp(i*theta(n,:)) * rfft(x[b,n,:]), n=D)
where theta(n,f) = 2*(p0[n]*Im(rfft(circ0))[f] + p1[n]*Im(rfft(circ1))[f]).

Sharding: data-parallel over batch, 4 batches per core (8 cores).

Host prep (inside kernel(), per core) — O(input) data prep; all DFT
matmul math runs on device:
  - two-level even/odd fold of x (radix-2 DIF twice) -> eo2 (768 cols)
  - permute to the exact per-tile (partition, chunk*row) layout so each
    tile load is one dense contiguous (128, 3072) fp16 DMA
  - cos/sin phase tables cos(theta)/sin(theta) in slot layout, fp16
  - packed block-sparse constant matrices: f2p = the 18 nonzero
    128x128 blocks of the L2-folded forward DFT; g2p = the u/v blocks
    of the folded inverse.

Device per (batch, 512-row half):
  - fwd: 18 fp16 matmuls (moving = xtb chunks, N=512) -> PSUM;
    PSUM->SBUF fp16 copies split ACT (j=0,1) / DVE (j=2)
  - rotation: 6 fp16 2x tensor ops split DVE (3) / Pool (3)
  - inverse: 7 fp16 matmuls per 128-row group -> u (385)/v (383) PSUM;
    reversed PSUM->SBUF fp16 copies split ACT (3) / DVE (1);
    un-fold split gpsimd (lo-sub) / DVE (hi-add); two half-tile
    (128, 1536) stores to a permuted DRAM layout, un-permuted on host.

DMA rings: constants/tables on the ACT HWDGE ring, x loads / out
stores on the SP ring, so tile streaming starts at t=0.
"""
import math
from contextlib import ExitStack

import numpy as np

import concourse.bacc as bacc
import concourse.tile as tile
from concourse import mybir
from concourse import bass_utils

F32 = mybir.dt.float32
F16 = mybir.dt.float16

B, N, D = 32, 1024, 768
NCORES = 8
BS = B // NCORES
P = 128
NCH = D // P              # 6
ROWTILE = 512
NG = ROWTILE // P         # 4

# forward block list (v4 slot/col layout): M-chunk -> list of K-chunks.
# The spare I-slot 384 (f=384 cos row, the only nonzero of chunk 3 in
# d2-chunks 0/1) is host-injected (z384), so j=3 keeps only its sin
# blocks [4, 5].
FWD_BLOCKS = {0: [0, 1], 1: [1, 2, 3], 2: [0, 1, 2, 3],
              3: [4, 5], 4: [3, 4], 5: [3, 4, 5]}
# packed col offset of forward block (j, c) in f2p
FOFF = {}
_off = 0
for _j in range(6):
    for _c in FWD_BLOCKS[_j]:
        FOFF[(_j, _c)] = _off
        _off += P
NFB = _off // P           # 18
# packed col offsets of inverse u (c=0..3, 386 wide) / v (c=3..5, 385)
UOFF = [c * 386 for c in range(4)]
VOFF = [4 * 386 + (c - 3) * 385 for c in (3, 4, 5)]
GW = 4 * 386 + 3 * 385    # 2699


# ---------------- host-side constants (L2-folded DFT) ----------------

def _slot_f_map():
    f = np.zeros(384, dtype=np.int64)
    f[0:128] = 2 * np.arange(128)
    f[128:256] = 2 * np.arange(128) + 1
    f[256:320] = 256 + 2 * np.arange(64)
    f[320:384] = 257 + 2 * np.arange(64)
    return f


def _build_f2g2():
    fmap = _slot_f_map()
    d2 = np.arange(193)
    F2 = np.zeros((768, 768), dtype=np.float64)
    for s in range(384):
        f = fmap[s]
        if f % 2 == 0:
            F2[s, 0:193] = np.cos(2 * np.pi * f * d2 / D)
        else:
            F2[s, 193:385] = np.cos(2 * np.pi * f * np.arange(192) / D)
    F2[384, 0:193] = np.cos(2 * np.pi * 384 * d2 / D)
    for s in range(1, 384):
        f = fmap[s]
        if f % 2 == 0:
            F2[384 + s, 577:768] = -np.sin(
                2 * np.pi * f * np.arange(1, 192) / D)
        else:
            F2[384 + s, 385:577] = -np.sin(
                2 * np.pi * f * np.arange(1, 193) / D)

    G2 = np.zeros((768, 772), dtype=np.float64)
    dd = np.arange(385)
    for s in range(384):
        f = fmap[s]
        w = (1.0 if f == 0 else 2.0) / D
        G2[s, 0:385] = w * np.cos(2 * np.pi * f * dd / D)
    G2[384, 0:385] = (1.0 / D) * np.cos(2 * np.pi * 384 * dd / D)
    dv = np.arange(1, 384)
    for s in range(1, 384):
        f = fmap[s]
        G2[384 + s, 385 + dv] = (2.0 / D) * np.sin(2 * np.pi * f * dv / D)
    return F2, G2


def _build_matrices():
    """Packed nonzero blocks: f2p (128, 16*128), g2p (128, 2699) fp16."""
    F2, G2 = _build_f2g2()
    F2 = F2.copy()
    F2[384, :] = 0.0          # spare row host-injected via z384
    F2T = F2.T
    fb = [F2T[c * P:(c + 1) * P, j * P:(j + 1) * P]
          for j in range(6) for c in FWD_BLOCKS[j]]
    f2p = np.ascontiguousarray(
        np.concatenate(fb, axis=1)).astype(np.float16)
    gb = [G2[c * P:(c + 1) * P, 0:386] for c in range(4)]
    gb += [G2[c * P:(c + 1) * P, 386:771] for c in (3, 4, 5)]
    g2p = np.ascontiguousarray(
        np.concatenate(gb, axis=1)).astype(np.float16)
    return f2p, g2p


def _build_tables(circ, positions):
    """cos/sin(theta) tables, slot layout, fp16: (2, 128, 1536) each.

    tab[h][p, j*512 + nn] = f(theta(slot=j*128+p, n=h*512+nn)).
    """
    fmap = _slot_f_map()
    S = np.imag(np.fft.rfft(circ.astype(np.float64), axis=-1))  # (2, 385)
    Ss = S[:, fmap]                                             # (2, 384)
    pos = positions.astype(np.float64)                          # (N, 2)
    theta = 2.0 * (pos[:, 0][None, :] * Ss[0][:, None]
                   + pos[:, 1][None, :] * Ss[1][:, None])       # (384, N)

    def to_layout(a):  # (384, N) -> (2, 128, 1536)
        return np.ascontiguousarray(
            a.reshape(3, 128, 2, 512).transpose(2, 1, 0, 3).reshape(
                2, 128, 1536)).astype(np.float16)

    return to_layout(np.cos(theta)), to_layout(np.sin(theta))


def _fold2(x):
    """x (..., 768) fp32 -> eo2 (..., 768)."""
    e = np.zeros(x.shape[:-1] + (385,), dtype=x.dtype)
    e[..., 0] = x[..., 0]
    e[..., 384] = x[..., 384]
    e[..., 1:384] = x[..., 1:384] + x[..., 385:768][..., ::-1]
    o = np.zeros(x.shape[:-1] + (385,), dtype=x.dtype)
    o[..., 1:384] = x[..., 1:384] - x[..., 385:768][..., ::-1]
    eo2 = np.empty_like(x)
    eo2[..., 0] = e[..., 0] + e[..., 384]
    eo2[..., 1:192] = e[..., 1:192] + e[..., 193:384][..., ::-1]
    eo2[..., 192] = e[..., 192]
    eo2[..., 193] = e[..., 0] - e[..., 384]
    eo2[..., 194:385] = e[..., 1:192] - e[..., 193:384][..., ::-1]
    eo2[..., 385:576] = o[..., 1:192] + o[..., 193:384][..., ::-1]
    eo2[..., 576] = o[..., 192]
    eo2[..., 577:768] = o[..., 1:192] - o[..., 193:384][..., ::-1]
    return eo2


# ---------------- device kernel ----------------

def build_kernel(reps=1, trace_sim=False):
    nc = bacc.Bacc("TRN2", target_bir_lowering=False, debug=False,
                   num_devices=NCORES)
    xt = nc.dram_tensor("xt", [BS, 2, P, NCH * ROWTILE], F16,
                        kind="ExternalInput").ap()
    ctab_d = nc.dram_tensor("ctab", [2, P, 1536], F16,
                            kind="ExternalInput").ap()
    stab_d = nc.dram_tensor("stab", [2, P, 1536], F16,
                            kind="ExternalInput").ap()
    f2p_d = nc.dram_tensor("f2p", [P, NFB * P], F16,
                           kind="ExternalInput").ap()
    g2p_d = nc.dram_tensor("g2p", [P, GW], F16, kind="ExternalInput").ap()
    z384_d = nc.dram_tensor("z384", [P, BS * 2 * NG], F16,
                            kind="ExternalInput").ap()
    altc_d = nc.dram_tensor("altc", [P, 385], F16,
                            kind="ExternalInput").ap()
    out16 = nc.dram_tensor("out", [BS, 2, P, NG * D], F16,
                           kind="ExternalOutput").ap()

    with tile.TileContext(nc, trace_sim=trace_sim) as tc, ExitStack() as ctx:
        consts = ctx.enter_context(tc.tile_pool(name="consts", bufs=1))
        tabs = ctx.enter_context(tc.tile_pool(name="tabs", bufs=1))
        xio = ctx.enter_context(tc.tile_pool(name="xio", bufs=2))
        work = ctx.enter_context(tc.tile_pool(name="work", bufs=2))

        # ---- constants on the ACT HWDGE ring, ordered by first use ----
        fpP = consts.tile([P, NFB * P], F16, tag="fpP", name="fpP")
        # j=0 blocks (first two) land first so the first matmul can start
        nc.scalar.dma_start(out=fpP[:, 0:2 * P], in_=f2p_d[:, 0:2 * P])
        nc.scalar.dma_start(out=fpP[:, 2 * P:], in_=f2p_d[:, 2 * P:])
        cTb = [tabs.tile([P, 1536], F16, tag=f"cTb{h}", name=f"cTb{h}")
               for h in range(2)]
        sTb = [tabs.tile([P, 1536], F16, tag=f"sTb{h}", name=f"sTb{h}")
               for h in range(2)]
        nc.scalar.dma_start(out=cTb[0], in_=ctab_d[0])
        nc.scalar.dma_start(out=sTb[0], in_=stab_d[0])
        gpP = consts.tile([P, GW], F16, tag="gpP", name="gpP")
        # later-needed constants go via the gpsimd SWDGE ring; the Pool
        # engine is idle until the first rotation, so these are free and
        # keep the ACT HWDGE ring clear for the first PSUM->SBUF copies.
        nc.gpsimd.dma_start(out=gpP, in_=g2p_d)
        nc.gpsimd.dma_start(out=cTb[1], in_=ctab_d[1])
        nc.gpsimd.dma_start(out=sTb[1], in_=stab_d[1])
        # host-computed f=384 bin, transposed to (n-partition, tile*group)
        # so it can be a per-partition scalar in the inverse, where it
        # enters the folded u via u += z384 * (-1)^k / 768 (one Pool
        # scalar_tensor_tensor per group replaces a 386-col matmul)
        zT = tabs.tile([P, BS * 2 * NG], F16, tag="zT", name="zT")
        nc.gpsimd.dma_start(out=zT, in_=z384_d)
        altc = tabs.tile([P, 385], F16, tag="altc", name="altc")
        nc.gpsimd.dma_start(out=altc, in_=altc_d)

        # ---- main loop (software-pipelined 2 deep: PE runs fwd(k)
        # then inv(k-2), so the rotation of tile k-1 overlaps fwd(k)) ----
        psf = ctx.enter_context(tc.tile_pool(name="psf", bufs=2, space="PSUM"))
        psi = ctx.enter_context(tc.tile_pool(name="psi", bufs=2, space="PSUM"))

        tiles = [(b, h) for _ in range(reps) for b in range(BS)
                 for h in range(2)]
        nt = len(tiles)
        st = {}   # per-tile live state: riR/riI for pending inverse
        lds = {}  # per-tile prefetched xtb

        def issue_load(k, split=False):
            b, h = tiles[k]
            xtb = xio.tile([P, NCH * ROWTILE], F16, tag="xtb", bufs=3)
            if split:
                # chunks {0,1} land first so fwd j=0 can start sooner
                nc.sync.dma_start(out=xtb[:, 0:2 * ROWTILE],
                                  in_=xt[b, h][:, 0:2 * ROWTILE])
                nc.sync.dma_start(out=xtb[:, 2 * ROWTILE:],
                                  in_=xt[b, h][:, 2 * ROWTILE:])
            else:
                nc.sync.dma_start(out=xtb, in_=xt[b, h])
            lds[k] = xtb

        def fwd_j(k, xtb, xRI, j):
            pf = psf.tile([P, 1024], F32, tag="psf", name="pf")
            pR = pf[:, 0:512]
            pI = pf[:, 512:1024]
            kR = FWD_BLOCKS[j]
            for i, c in enumerate(kR):
                o = FOFF[(j, c)]
                nc.tensor.matmul(pR, fpP[:, o:o + P],
                                 xtb[:, c * ROWTILE:(c + 1) * ROWTILE],
                                 start=(i == 0), stop=(i == len(kR) - 1))
            kI = FWD_BLOCKS[3 + j]
            for i, c in enumerate(kI):
                o = FOFF[(3 + j, c)]
                nc.tensor.matmul(pI, fpP[:, o:o + P],
                                 xtb[:, c * ROWTILE:(c + 1) * ROWTILE],
                                 start=(i == 0), stop=(i == len(kI) - 1))
            dst = xRI.rearrange("p (k q) -> p k q",
                                k=2)[:, :, j * 512:(j + 1) * 512]
            src = pf.rearrange("p (k q) -> p k q", k=2)
            if j < 2:
                nc.scalar.copy(out=dst, in_=src)
            else:
                nc.vector.tensor_copy(out=dst, in_=src)

        def inv_g(k, riR, riI, osb, g, zci):
            def ri_slice(c):
                if c < 3:
                    return riR[:, c * 512 + g * P: c * 512 + (g + 1) * P]
                return riI[:, (c - 3) * 512 + g * P:
                           (c - 3) * 512 + (g + 1) * P]

            # inverse (folded): u (385) / v (383+pad) in one 2-bank psum
            # tile; merged reversed PSUM->SBUF copy per group into
            # uv = [u_384..u_0 | v-desc], so both un-fold inputs are
            # ascending for the DVE 2x hi-add; gpsimd takes the lo-sub.
            pi_ = psi.tile([P, 1024], F32, tag="pi", name="pi_")
            pa = pi_[:, 0:512]
            pb = pi_[:, 512:1024]
            for i, c in enumerate((0, 1, 2)):
                nc.tensor.matmul(pa[:, 0:386], ri_slice(c),
                                 gpP[:, UOFF[c]:UOFF[c] + 386],
                                 start=(i == 0), stop=(i == 2))
            for i, c in enumerate((3, 4, 5)):
                o = VOFF[c - 3]
                nc.tensor.matmul(pb[:, 0:385], ri_slice(c),
                                 gpP[:, o:o + 385],
                                 start=(i == 0), stop=(i == 2))
            # uv[k]=u_{384-k} (k=0..384), uv[770-d]=v_d (v at
            # cols 386..769 descending; uv[770] memset to 0)
            uv = work.tile([P, 772], F16, tag="uv", bufs=3)
            dst = uv[:, 0:770].rearrange("p (k d) -> p k d", k=2)
            src = pi_.rearrange("p (k d) -> p k d", k=2)[:, :, 384::-1]
            nc.scalar.copy(out=dst, in_=src)
            nc.vector.memset(uv[:, 770:772], 0.0)
            # u2[k] = u_{384-k} + z384[n] * (-1)^k / 768: the spare-row
            # (f=384) rank-1 term, with z384 as a per-partition scalar.
            # TensorScalarPtr (AP scalar) is DVE-only on HW.
            u2 = work.tile([P, 385], F16, tag="u2", bufs=3)
            nc.vector.scalar_tensor_tensor(
                u2, altc, zT[:, zci + g:zci + g + 1], uv[:, 0:385],
                op0=mybir.AluOpType.mult, op1=mybir.AluOpType.add)
            gs = g * D
            # lo: out[d] = u_d - v_d (d=0..383; d=0: v_0 slot = 0)
            nc.gpsimd.tensor_sub(osb[:, gs:gs + 384],
                                 u2[:, 384:0:-1], uv[:, 770:386:-1])
            # hi: out[384+m] = u_{384-m} + v_{384-m} (m=0..383)
            nc.vector.tensor_add(osb[:, gs + 384:gs + 768],
                                 u2[:, 0:384], uv[:, 386:770])

        def issue_iter(k):
            """Interleave inv(k-2) groups with fwd(k) j-blocks so the PE
            always has independent work while PSUM slots recycle."""
            front = k < nt
            back = k >= 2
            if front:
                b, h = tiles[k]
                xtb = lds.pop(k)
                xRI = work.tile([P, 3072], F16, tag="xRI", bufs=3)
            if back:
                bb, hb = tiles[k - 2]
                riR, riI = st.pop(k - 2)
                osb = xio.tile([P, NG * D], F16, tag="osb")
                zci = (bb * 2 + hb) * NG
            # PE stream: g0 g1 j0 g2 j1 g3 j2
            if back:
                inv_g(k - 2, riR, riI, osb, 0, zci)
                inv_g(k - 2, riR, riI, osb, 1, zci)
            if front:
                fwd_j(k, xtb, xRI, 0)
            if back:
                inv_g(k - 2, riR, riI, osb, 2, zci)
                nc.sync.dma_start(out=out16[bb, hb][:, 0:2 * D],
                                  in_=osb[:, 0:2 * D])
            if front:
                fwd_j(k, xtb, xRI, 1)
            if back:
                inv_g(k - 2, riR, riI, osb, 3, zci)
                nc.sync.dma_start(out=out16[bb, hb][:, 2 * D:3 * D],
                                  in_=osb[:, 2 * D:3 * D])
            if front:
                fwd_j(k, xtb, xRI, 2)
            if back:
                nc.sync.dma_start(out=out16[bb, hb][:, 3 * D:4 * D],
                                  in_=osb[:, 3 * D:4 * D])
            if front:
                # rotation: 6 fp16 2x tensor ops, split DVE (3) / Pool (3)
                t1 = work.tile([P, 1536], F16, tag="rt1")
                t2 = work.tile([P, 1536], F16, tag="rt2")
                t3 = work.tile([P, 1536], F16, tag="rt3")
                t4 = work.tile([P, 1536], F16, tag="rt4")
                riRn = work.tile([P, 1536], F16, tag="riR", bufs=3)
                riIn = work.tile([P, 1536], F16, tag="riI", bufs=3)
                xRb = xRI[:, 0:1536]
                xIb = xRI[:, 1536:3072]
                nc.vector.tensor_mul(t1, xRb, cTb[h])
                nc.vector.tensor_mul(t2, xIb, sTb[h])
                nc.gpsimd.tensor_mul(t3, xRb, sTb[h])
                nc.gpsimd.tensor_mul(t4, xIb, cTb[h])
                nc.vector.tensor_sub(riRn, t1, t2)
                nc.gpsimd.tensor_add(riIn, t3, t4)
                st[k] = (riRn, riIn)

        issue_load(0, split=True)
        issue_load(1, split=True)
        for k in range(nt + 2):
            if k + 2 < nt:
                issue_load(k + 2)
            issue_iter(k)
    nc.finalize()
    return nc


_NC_CACHE = {}


def _host_prep(x):
    """(BS, N, D) fp32 -> L2-folded (BS, 2, 128, 6*512) fp16 tile layout
    plus the f=384 bin z384 (1, BS*2*512) fp16.

    xtb[p, c*512+r] = eo2[b, h*512+r, c*128+p];
    z384[(b*2+h)*512+r] = sum_d (-1)^d x[b, h*512+r, d].
    """
    eo2 = _fold2(x)
    xt = eo2.reshape(BS, 2, ROWTILE, NCH, P).transpose(0, 1, 4, 3, 2)
    xt = np.ascontiguousarray(xt).astype(np.float16).reshape(
        BS, 2, P, NCH * ROWTILE)
    sgn = np.where(np.arange(D) % 2 == 0, 1.0, -1.0).astype(np.float32)
    z = (x @ sgn).reshape(BS, 2, NG, P)           # [b, h, g, p]
    z384 = np.ascontiguousarray(
        z.transpose(3, 0, 1, 2)).astype(np.float16).reshape(
        P, BS * 2 * NG)                           # [p, (b,h,g)]
    return xt, z384


def _host_post(res16):
    """(BS, 2, 128, 4*768) fp16 -> (BS, N, D) fp32.

    osb[p, g*768+d] = out[b, h*512+g*128+p, d].
    """
    r = res16.reshape(BS, 2, P, NG, D).transpose(0, 1, 3, 2, 4)
    return np.ascontiguousarray(r).astype(np.float32).reshape(BS, N, D)


def make_in_maps(inputs):
    x = np.ascontiguousarray(inputs["x"], dtype=np.float32)
    circ = np.ascontiguousarray(inputs["circ"], dtype=np.float32)
    positions = np.ascontiguousarray(inputs["positions"], dtype=np.int32)
    if "mats" not in _NC_CACHE:
        _NC_CACHE["mats"] = _build_matrices()
    f2p, g2p = _NC_CACHE["mats"]
    ctab, stab = _build_tables(circ, positions)
    altk = np.where(np.arange(385) % 2 == 0, 1.0, -1.0) / D
    altc = np.ascontiguousarray(
        np.broadcast_to(altk, (P, 385))).astype(np.float16)
    in_maps = []
    for core in range(NCORES):
        xt, z384 = _host_prep(x[core * BS:(core + 1) * BS])
        in_maps.append({
            "xt": xt,
            "z384": z384,
            "altc": altc,
            "ctab": ctab,
            "stab": stab,
            "f2p": f2p,
            "g2p": g2p,
        })
    return in_maps


def kernel(x, circ, positions):
    if "nc" not in _NC_CACHE:
        _NC_CACHE["nc"] = build_kernel()
    nc = _NC_CACHE["nc"]
    in_maps = make_in_maps({"x": x, "circ": circ, "positions": positions})
    res = bass_utils.run_bass_kernel_spmd(nc, in_maps,
                                          core_ids=list(range(NCORES)))
    out = np.concatenate(
        [_host_post(res.results[c]["out"]) for c in range(NCORES)], axis=0)
    return out


if __name__ == "__main__":
    rng = np.random.default_rng(0)
    x = rng.standard_normal((B, N, D)).astype(np.float32)
    circ = (rng.standard_normal((2, D)) * 0.01).astype(np.float32)
    positions = rng.integers(0, 32, (N, 2)).astype(np.int32)
    out = kernel(x=x, circ=circ, positions=positions)
    print("out", out.shape, out.dtype)


# revision 27
# speedup vs baseline: 1.0296x; 1.0296x over previous
"""Trainium2 Bass kernel for nn_CirculantSTRING (v7).

Math: out[b,n,:] = irfft(exp(i*theta(n,:)) * rfft(x[b,n,:]), n=D)
where theta(n,f) = 2*(p0[n]*Im(rfft(circ0))[f] + p1[n]*Im(rfft(circ1))[f]).

Sharding: data-parallel over batch, 4 batches per core (8 cores).

Host prep (inside kernel(), per core) — O(input) data prep; all DFT
matmul math runs on device:
  - two-level even/odd fold of x (radix-2 DIF twice) -> eo2 (768 cols)
  - permute to the exact per-tile (partition, chunk*row) layout so each
    tile load is one dense contiguous (128, 3072) fp16 DMA
  - cos/sin phase tables cos(theta)/sin(theta) in slot layout, fp16
  - packed block-sparse constant matrices: f2p = the 16 nonzero
    128x128 blocks of the L2-folded forward DFT (the spare f=384 cos
    row is zeroed out); g2p = the u/v blocks of the folded inverse
    with the u sum trimmed to slot chunks 0-2
  - z384[n] = sum_d (-1)^d x[n,d] (the f=384 bin), shipped transposed
    so it enters the inverse as a per-partition scalar.

Device per (batch, 512-row half):
  - fwd: 16 fp16 matmuls (moving = xtb chunks, N=512) -> PSUM;
    PSUM->SBUF fp16 copies split ACT (j=0,1) / DVE (j=2)
  - rotation: 6 fp16 2x tensor ops split DVE (3) / Pool (3)
  - inverse: 6 fp16 matmuls per 128-row group -> u (386)/v (385) PSUM;
    merged reversed PSUM->SBUF fp16 copy on ACT; the spare-row rank-1
    term added via DVE scalar_tensor_tensor (u2 = altc*z384 + u);
    un-fold split gpsimd (lo-sub) / DVE (hi-add); 3 partial stores to
    a permuted DRAM layout, un-permuted on host.

DMA rings: early constants on the ACT HWDGE ring, later ones on the
gpsimd SWDGE ring, x loads / out stores on the SP ring, so tile
streaming starts at t=0 and the first matmul fires at ~2.5us.
"""
from contextlib import ExitStack

import numpy as np

import concourse.bacc as bacc
import concourse.tile as tile
from concourse import mybir
from concourse import bass_utils

F32 = mybir.dt.float32
F16 = mybir.dt.float16

B, N, D = 32, 1024, 768
NCORES = 8
BS = B // NCORES
P = 128
NCH = D // P              # 6
ROWTILE = 512
NG = ROWTILE // P         # 4

# forward block list (v4 slot/col layout): M-chunk -> list of K-chunks.
# The spare I-slot 384 (f=384 cos row, the only nonzero of chunk 3 in
# d2-chunks 0/1) is host-injected (z384), so j=3 keeps only its sin
# blocks [4, 5].
FWD_BLOCKS = {0: [0, 1], 1: [1, 2, 3], 2: [0, 1, 2, 3],
              3: [4, 5], 4: [3, 4], 5: [3, 4, 5]}
# packed col offset of forward block (j, c) in f2p
FOFF = {}
_off = 0
for _j in range(6):
    for _c in FWD_BLOCKS[_j]:
        FOFF[(_j, _c)] = _off
        _off += P
NFB = _off // P           # 18
# packed col offsets of inverse u (c=0..3, 386 wide) / v (c=3..5, 385)
UOFF = [c * 386 for c in range(4)]
VOFF = [4 * 386 + (c - 3) * 385 for c in (3, 4, 5)]
GW = 4 * 386 + 3 * 385    # 2699


# ---------------- host-side constants (L2-folded DFT) ----------------

def _slot_f_map():
    f = np.zeros(384, dtype=np.int64)
    f[0:128] = 2 * np.arange(128)
    f[128:256] = 2 * np.arange(128) + 1
    f[256:320] = 256 + 2 * np.arange(64)
    f[320:384] = 257 + 2 * np.arange(64)
    return f


def _build_f2g2():
    fmap = _slot_f_map()
    d2 = np.arange(193)
    F2 = np.zeros((768, 768), dtype=np.float64)
    for s in range(384):
        f = fmap[s]
        if f % 2 == 0:
            F2[s, 0:193] = np.cos(2 * np.pi * f * d2 / D)
        else:
            F2[s, 193:385] = np.cos(2 * np.pi * f * np.arange(192) / D)
    F2[384, 0:193] = np.cos(2 * np.pi * 384 * d2 / D)
    for s in range(1, 384):
        f = fmap[s]
        if f % 2 == 0:
            F2[384 + s, 577:768] = -np.sin(
                2 * np.pi * f * np.arange(1, 192) / D)
        else:
            F2[384 + s, 385:577] = -np.sin(
                2 * np.pi * f * np.arange(1, 193) / D)

    G2 = np.zeros((768, 772), dtype=np.float64)
    dd = np.arange(385)
    for s in range(384):
        f = fmap[s]
        w = (1.0 if f == 0 else 2.0) / D
        G2[s, 0:385] = w * np.cos(2 * np.pi * f * dd / D)
    G2[384, 0:385] = (1.0 / D) * np.cos(2 * np.pi * 384 * dd / D)
    dv = np.arange(1, 384)
    for s in range(1, 384):
        f = fmap[s]
        G2[384 + s, 385 + dv] = (2.0 / D) * np.sin(2 * np.pi * f * dv / D)
    return F2, G2


def _build_matrices():
    """Packed nonzero blocks: f2p (128, 16*128), g2p (128, 2699) fp16."""
    F2, G2 = _build_f2g2()
    F2 = F2.copy()
    F2[384, :] = 0.0          # spare row host-injected via z384
    F2T = F2.T
    fb = [F2T[c * P:(c + 1) * P, j * P:(j + 1) * P]
          for j in range(6) for c in FWD_BLOCKS[j]]
    f2p = np.ascontiguousarray(
        np.concatenate(fb, axis=1)).astype(np.float16)
    gb = [G2[c * P:(c + 1) * P, 0:386] for c in range(4)]
    gb += [G2[c * P:(c + 1) * P, 386:771] for c in (3, 4, 5)]
    g2p = np.ascontiguousarray(
        np.concatenate(gb, axis=1)).astype(np.float16)
    return f2p, g2p


def _build_tables(circ, positions):
    """cos/sin(theta) tables, slot layout, fp16: (2, 128, 1536) each.

    tab[h][p, j*512 + nn] = f(theta(slot=j*128+p, n=h*512+nn)).
    """
    fmap = _slot_f_map()
    S = np.imag(np.fft.rfft(circ.astype(np.float64), axis=-1))  # (2, 385)
    Ss = S[:, fmap]                                             # (2, 384)
    pos = positions.astype(np.float64)                          # (N, 2)
    theta = 2.0 * (pos[:, 0][None, :] * Ss[0][:, None]
                   + pos[:, 1][None, :] * Ss[1][:, None])       # (384, N)

    def to_layout(a):  # (384, N) -> (2, 128, 1536)
        return np.ascontiguousarray(
            a.reshape(3, 128, 2, 512).transpose(2, 1, 0, 3).reshape(
                2, 128, 1536)).astype(np.float16)

    return to_layout(np.cos(theta)), to_layout(np.sin(theta))


def _fold2(x):
    """x (..., 768) fp32 -> eo2 (..., 768)."""
    e = np.zeros(x.shape[:-1] + (385,), dtype=x.dtype)
    e[..., 0] = x[..., 0]
    e[..., 384] = x[..., 384]
    e[..., 1:384] = x[..., 1:384] + x[..., 385:768][..., ::-1]
    o = np.zeros(x.shape[:-1] + (385,), dtype=x.dtype)
    o[..., 1:384] = x[..., 1:384] - x[..., 385:768][..., ::-1]
    eo2 = np.empty_like(x)
    eo2[..., 0] = e[..., 0] + e[..., 384]
    eo2[..., 1:192] = e[..., 1:192] + e[..., 193:384][..., ::-1]
    eo2[..., 192] = e[..., 192]
    eo2[..., 193] = e[..., 0] - e[..., 384]
    eo2[..., 194:385] = e[..., 1:192] - e[..., 193:384][..., ::-1]
    eo2[..., 385:576] = o[..., 1:192] + o[..., 193:384][..., ::-1]
    eo2[..., 576] = o[..., 192]
    eo2[..., 577:768] = o[..., 1:192] - o[..., 193:384][..., ::-1]
    return eo2


# ---------------- device kernel ----------------

def build_kernel(reps=1, trace_sim=False):
    nc = bacc.Bacc("TRN2", target_bir_lowering=False, debug=False,
                   num_devices=NCORES)
    xt = nc.dram_tensor("xt", [BS, 2, P, NCH * ROWTILE], F16,
                        kind="ExternalInput").ap()
    ctab_d = nc.dram_tensor("ctab", [2, P, 1536], F16,
                            kind="ExternalInput").ap()
    stab_d = nc.dram_tensor("stab", [2, P, 1536], F16,
                            kind="ExternalInput").ap()
    f2p_d = nc.dram_tensor("f2p", [P, NFB * P], F16,
                           kind="ExternalInput").ap()
    g2p_d = nc.dram_tensor("g2p", [P, GW], F16, kind="ExternalInput").ap()
    z384_d = nc.dram_tensor("z384", [P, BS * 2 * NG], F16,
                            kind="ExternalInput").ap()
    altc_d = nc.dram_tensor("altc", [P, 385], F16,
                            kind="ExternalInput").ap()
    out16 = nc.dram_tensor("out", [BS, 2, P, NG * D], F16,
                           kind="ExternalOutput").ap()

    with tile.TileContext(nc, trace_sim=trace_sim) as tc, ExitStack() as ctx:
        consts = ctx.enter_context(tc.tile_pool(name="consts", bufs=1))
        tabs = ctx.enter_context(tc.tile_pool(name="tabs", bufs=1))
        xio = ctx.enter_context(tc.tile_pool(name="xio", bufs=2))
        work = ctx.enter_context(tc.tile_pool(name="work", bufs=2))

        # ---- constants on the ACT HWDGE ring, ordered by first use ----
        fpP = consts.tile([P, NFB * P], F16, tag="fpP", name="fpP")
        # j=0 blocks (first two) land first so the first matmul can start
        nc.scalar.dma_start(out=fpP[:, 0:2 * P], in_=f2p_d[:, 0:2 * P])
        nc.scalar.dma_start(out=fpP[:, 2 * P:], in_=f2p_d[:, 2 * P:])
        cTb = [tabs.tile([P, 1536], F16, tag=f"cTb{h}", name=f"cTb{h}")
               for h in range(2)]
        sTb = [tabs.tile([P, 1536], F16, tag=f"sTb{h}", name=f"sTb{h}")
               for h in range(2)]
        nc.scalar.dma_start(out=cTb[0], in_=ctab_d[0])
        nc.scalar.dma_start(out=sTb[0], in_=stab_d[0])
        gpP = consts.tile([P, GW], F16, tag="gpP", name="gpP")
        # later-needed constants go via the gpsimd SWDGE ring; the Pool
        # engine is idle until the first rotation, so these are free and
        # keep the ACT HWDGE ring clear for the first PSUM->SBUF copies.
        nc.gpsimd.dma_start(out=gpP, in_=g2p_d)
        nc.gpsimd.dma_start(out=cTb[1], in_=ctab_d[1])
        nc.gpsimd.dma_start(out=sTb[1], in_=stab_d[1])
        # host-computed f=384 bin, transposed to (n-partition, tile*group)
        # so it can be a per-partition scalar in the inverse, where it
        # enters the folded u via u += z384 * (-1)^k / 768 (one Pool
        # scalar_tensor_tensor per group replaces a 386-col matmul)
        zT = tabs.tile([P, BS * 2 * NG], F16, tag="zT", name="zT")
        nc.gpsimd.dma_start(out=zT, in_=z384_d)
        altc = tabs.tile([P, 385], F16, tag="altc", name="altc")
        nc.gpsimd.dma_start(out=altc, in_=altc_d)

        # ---- main loop (software-pipelined 2 deep: PE runs fwd(k)
        # then inv(k-2), so the rotation of tile k-1 overlaps fwd(k)) ----
        psf = ctx.enter_context(tc.tile_pool(name="psf", bufs=2, space="PSUM"))
        psi = ctx.enter_context(tc.tile_pool(name="psi", bufs=2, space="PSUM"))

        tiles = [(b, h) for _ in range(reps) for b in range(BS)
                 for h in range(2)]
        nt = len(tiles)
        st = {}   # per-tile live state: riR/riI for pending inverse
        lds = {}  # per-tile prefetched xtb

        def issue_load(k, split=False):
            b, h = tiles[k]
            xtb = xio.tile([P, NCH * ROWTILE], F16, tag="xtb", bufs=3)
            if split:
                # chunks {0,1} land first so fwd j=0 can start sooner
                nc.sync.dma_start(out=xtb[:, 0:2 * ROWTILE],
                                  in_=xt[b, h][:, 0:2 * ROWTILE])
                nc.sync.dma_start(out=xtb[:, 2 * ROWTILE:],
                                  in_=xt[b, h][:, 2 * ROWTILE:])
            else:
                nc.sync.dma_start(out=xtb, in_=xt[b, h])
            lds[k] = xtb

        def fwd_j(k, xtb, xRI, j):
            pf = psf.tile([P, 1024], F32, tag="psf", name="pf")
            pR = pf[:, 0:512]
            pI = pf[:, 512:1024]
            kR = FWD_BLOCKS[j]
            for i, c in enumerate(kR):
                o = FOFF[(j, c)]
                nc.tensor.matmul(pR, fpP[:, o:o + P],
                                 xtb[:, c * ROWTILE:(c + 1) * ROWTILE],
                                 start=(i == 0), stop=(i == len(kR) - 1))
            kI = FWD_BLOCKS[3 + j]
            for i, c in enumerate(kI):
                o = FOFF[(3 + j, c)]
                nc.tensor.matmul(pI, fpP[:, o:o + P],
                                 xtb[:, c * ROWTILE:(c + 1) * ROWTILE],
                                 start=(i == 0), stop=(i == len(kI) - 1))
            dst = xRI.rearrange("p (k q) -> p k q",
                                k=2)[:, :, j * 512:(j + 1) * 512]
            src = pf.rearrange("p (k q) -> p k q", k=2)
            if j < 2:
                nc.scalar.copy(out=dst, in_=src)
            else:
                nc.vector.tensor_copy(out=dst, in_=src)

        def inv_g(k, riR, riI, osb, g, zci):
            def ri_slice(c):
                if c < 3:
                    return riR[:, c * 512 + g * P: c * 512 + (g + 1) * P]
                return riI[:, (c - 3) * 512 + g * P:
                           (c - 3) * 512 + (g + 1) * P]

            # inverse (folded): u (385) / v (383+pad) in one 2-bank psum
            # tile; merged reversed PSUM->SBUF copy per group into
            # uv = [u_384..u_0 | v-desc], so both un-fold inputs are
            # ascending for the DVE 2x hi-add; gpsimd takes the lo-sub.
            pi_ = psi.tile([P, 1024], F32, tag="pi", name="pi_")
            pa = pi_[:, 0:512]
            pb = pi_[:, 512:1024]
            for i, c in enumerate((0, 1, 2)):
                nc.tensor.matmul(pa[:, 0:386], ri_slice(c),
                                 gpP[:, UOFF[c]:UOFF[c] + 386],
                                 start=(i == 0), stop=(i == 2))
            for i, c in enumerate((3, 4, 5)):
                o = VOFF[c - 3]
                nc.tensor.matmul(pb[:, 0:385], ri_slice(c),
                                 gpP[:, o:o + 385],
                                 start=(i == 0), stop=(i == 2))
            # uv[k]=u_{384-k} (k=0..384), uv[770-d]=v_d (v at
            # cols 386..769 descending; uv[770] memset to 0)
            uv = work.tile([P, 772], F16, tag="uv", bufs=3)
            dst = uv[:, 0:770].rearrange("p (k d) -> p k d", k=2)
            src = pi_.rearrange("p (k d) -> p k d", k=2)[:, :, 384::-1]
            nc.scalar.copy(out=dst, in_=src)
            nc.vector.memset(uv[:, 770:772], 0.0)
            # u2[k] = u_{384-k} + z384[n] * (-1)^k / 768: the spare-row
            # (f=384) rank-1 term, with z384 as a per-partition scalar.
            # TensorScalarPtr (AP scalar) is DVE-only on HW.
            u2 = work.tile([P, 385], F16, tag="u2", bufs=3)
            nc.vector.scalar_tensor_tensor(
                u2, altc, zT[:, zci + g:zci + g + 1], uv[:, 0:385],
                op0=mybir.AluOpType.mult, op1=mybir.AluOpType.add)
            gs = g * D
            # lo: out[d] = u_d - v_d (d=0..383; d=0: v_0 slot = 0)
            nc.gpsimd.tensor_sub(osb[:, gs:gs + 384],
                                 u2[:, 384:0:-1], uv[:, 770:386:-1])
            # hi: out[384+m] = u_{384-m} + v_{384-m} (m=0..383)
            nc.vector.tensor_add(osb[:, gs + 384:gs + 768],
                                 u2[:, 0:384], uv[:, 386:770])

        def issue_iter(k):
            """Interleave inv(k-2) groups with fwd(k) j-blocks so the PE
            always has independent work while PSUM slots recycle."""
            front = k < nt
            back = k >= 2
            if front:
                b, h = tiles[k]
                xtb = lds.pop(k)
                xRI = work.tile([P, 3072], F16, tag="xRI", bufs=3)
            if back:
                bb, hb = tiles[k - 2]
                riR, riI = st.pop(k - 2)
                osb = xio.tile([P, NG * D], F16, tag="osb")
                zci = (bb * 2 + hb) * NG
            # PE stream: g0 g1 j0 g2 j1 g3 j2
            if back:
                inv_g(k - 2, riR, riI, osb, 0, zci)
                inv_g(k - 2, riR, riI, osb, 1, zci)
            if front:
                fwd_j(k, xtb, xRI, 0)
            if back:
                inv_g(k - 2, riR, riI, osb, 2, zci)
                nc.sync.dma_start(out=out16[bb, hb][:, 0:2 * D],
                                  in_=osb[:, 0:2 * D])
            if front:
                fwd_j(k, xtb, xRI, 1)
            if back:
                inv_g(k - 2, riR, riI, osb, 3, zci)
                nc.sync.dma_start(out=out16[bb, hb][:, 2 * D:3 * D],
                                  in_=osb[:, 2 * D:3 * D])
            if front:
                fwd_j(k, xtb, xRI, 2)
            if back:
                nc.sync.dma_start(out=out16[bb, hb][:, 3 * D:4 * D],
                                  in_=osb[:, 3 * D:4 * D])
            if front:
                # rotation: 6 fp16 2x tensor ops, split DVE (3) / Pool (3)
                t1 = work.tile([P, 1536], F16, tag="rt1")
                t2 = work.tile([P, 1536], F16, tag="rt2")
                t3 = work.tile([P, 1536], F16, tag="rt3")
                t4 = work.tile([P, 1536], F16, tag="rt4")
                riRn = work.tile([P, 1536], F16, tag="riR", bufs=3)
                riIn = work.tile([P, 1536], F16, tag="riI", bufs=3)
                xRb = xRI[:, 0:1536]
                xIb = xRI[:, 1536:3072]
                nc.vector.tensor_mul(t1, xRb, cTb[h])
                nc.vector.tensor_mul(t2, xIb, sTb[h])
                nc.gpsimd.tensor_mul(t3, xRb, sTb[h])
                nc.gpsimd.tensor_mul(t4, xIb, cTb[h])
                nc.vector.tensor_sub(riRn, t1, t2)
                nc.gpsimd.tensor_add(riIn, t3, t4)
                st[k] = (riRn, riIn)

        issue_load(0, split=True)
        issue_load(1, split=True)
        for k in range(nt + 2):
            if k + 2 < nt:
                issue_load(k + 2)
            issue_iter(k)
    nc.finalize()
    return nc


_NC_CACHE = {}


def _host_prep(x):
    """(BS, N, D) fp32 -> L2-folded (BS, 2, 128, 6*512) fp16 tile layout
    plus the f=384 bin z384 (1, BS*2*512) fp16.

    xtb[p, c*512+r] = eo2[b, h*512+r, c*128+p];
    z384[(b*2+h)*512+r] = sum_d (-1)^d x[b, h*512+r, d].
    """
    eo2 = _fold2(x)
    xt = eo2.reshape(BS, 2, ROWTILE, NCH, P).transpose(0, 1, 4, 3, 2)
    xt = np.ascontiguousarray(xt).astype(np.float16).reshape(
        BS, 2, P, NCH * ROWTILE)
    sgn = np.where(np.arange(D) % 2 == 0, 1.0, -1.0).astype(np.float32)
    z = (x @ sgn).reshape(BS, 2, NG, P)           # [b, h, g, p]
    z384 = np.ascontiguousarray(
        z.transpose(3, 0, 1, 2)).astype(np.float16).reshape(
        P, BS * 2 * NG)                           # [p, (b,h,g)]
    return xt, z384


def _host_post(res16):
    """(BS, 2, 128, 4*768) fp16 -> (BS, N, D) fp32.

    osb[p, g*768+d] = out[b, h*512+g*128+p, d].
    """
    r = res16.reshape(BS, 2, P, NG, D).transpose(0, 1, 3, 2, 4)
    return np.ascontiguousarray(r).astype(np.float32).reshape(BS, N, D)


def make_in_maps(inputs):
    x = np.ascontiguousarray(inputs["x"], dtype=np.float32)
    circ = np.ascontiguousarray(inputs["circ"], dtype=np.float32)
    positions = np.ascontiguousarray(inputs["positions"], dtype=np.int32)
    if "mats" not in _NC_CACHE:
        _NC_CACHE["mats"] = _build_matrices()
    f2p, g2p = _NC_CACHE["mats"]
    ctab, stab = _build_tables(circ, positions)
    altk = np.where(np.arange(385) % 2 == 0, 1.0, -1.0) / D
    altc = np.ascontiguousarray(
        np.broadcast_to(altk, (P, 385))).astype(np.float16)
    in_maps = []
    for core in range(NCORES):
        xt, z384 = _host_prep(x[core * BS:(core + 1) * BS])
        in_maps.append({
            "xt": xt,
            "z384": z384,
            "altc": altc,
            "ctab": ctab,
            "stab": stab,
            "f2p": f2p,
            "g2p": g2p,
        })
    return in_maps


def kernel(x, circ, positions):
    if "nc" not in _NC_CACHE:
        _NC_CACHE["nc"] = build_kernel()
    nc = _NC_CACHE["nc"]
    in_maps = make_in_maps({"x": x, "circ": circ, "positions": positions})
    res = bass_utils.run_bass_kernel_spmd(nc, in_maps,
                                          core_ids=list(range(NCORES)))
    out = np.concatenate(
        [_host_post(res.results[c]["out"]) for c in range(NCORES)], axis=0)
    return out


if __name__ == "__main__":
    rng = np.random.default_rng(0)
    x = rng.standard_normal((B, N, D)).astype(np.float32)
    circ = (rng.standard_normal((2, D)) * 0.01).astype(np.float32)
    positions = rng.integers(0, 32, (N, 2)).astype(np.int32)
    out = kernel(x=x, circ=circ, positions=positions)
    print("out", out.shape, out.dtype)


# revision 33
# speedup vs baseline: 1.0392x; 1.0094x over previous
"""Trainium2 Bass kernel for nn_CirculantSTRING (v7).

Math: out[b,n,:] = irfft(exp(i*theta(n,:)) * rfft(x[b,n,:]), n=D)
where theta(n,f) = 2*(p0[n]*Im(rfft(circ0))[f] + p1[n]*Im(rfft(circ1))[f]).

Sharding: data-parallel over batch, 4 batches per core (8 cores).

Host prep (inside kernel(), per core) — O(input) data prep; all DFT
matmul math runs on device:
  - two-level even/odd fold of x (radix-2 DIF twice) -> eo2 (768 cols)
  - permute to the exact per-tile (partition, chunk*row) layout so each
    tile load is one dense contiguous (128, 3072) fp16 DMA
  - cos/sin phase tables cos(theta)/sin(theta) in slot layout, fp16
  - packed block-sparse constant matrices: f2p = the 16 nonzero
    128x128 blocks of the L2-folded forward DFT (the spare f=384 cos
    row is zeroed out); g2p = the u/v blocks of the folded inverse
    with the u sum trimmed to slot chunks 0-2
  - z384[n] = sum_d (-1)^d x[n,d] (the f=384 bin), shipped transposed
    so it enters the inverse as a per-partition scalar.

Device per (batch, 512-row half):
  - fwd: 16 fp16 matmuls (moving = xtb chunks, N=512) -> PSUM;
    PSUM->SBUF fp16 copies split ACT (j=0,1) / DVE (j=2)
  - rotation: 6 fp16 2x tensor ops split DVE (3) / Pool (3)
  - inverse: 6 fp16 matmuls per 128-row group -> u (386)/v (385) PSUM;
    merged reversed PSUM->SBUF fp16 copy on ACT; the spare-row rank-1
    term added via DVE scalar_tensor_tensor (u2 = altc*z384 + u);
    un-fold split gpsimd (lo-sub) / DVE (hi-add); 3 partial stores to
    a permuted DRAM layout, un-permuted on host.

DMA rings: early constants on the ACT HWDGE ring, later ones on the
gpsimd SWDGE ring, x loads / out stores on the SP ring, so tile
streaming starts at t=0 and the first matmul fires at ~2.5us.
"""
from contextlib import ExitStack

import numpy as np

import concourse.bacc as bacc
import concourse.tile as tile
from concourse import mybir
from concourse import bass_utils

F32 = mybir.dt.float32
F16 = mybir.dt.float16

B, N, D = 32, 1024, 768
NCORES = 8
BS = B // NCORES
P = 128
NCH = D // P              # 6
ROWTILE = 512
NG = ROWTILE // P         # 4

# forward block list (v4 slot/col layout): M-chunk -> list of K-chunks.
# The spare I-slot 384 (f=384 cos row, the only nonzero of chunk 3 in
# d2-chunks 0/1) is host-injected (z384), so j=3 keeps only its sin
# blocks [4, 5].
FWD_BLOCKS = {0: [0, 1], 1: [1, 2, 3], 2: [0, 1, 2, 3],
              3: [4, 5], 4: [3, 4], 5: [3, 4, 5]}
# packed col offset of forward block (j, c) in f2p
FOFF = {}
_off = 0
for _j in range(6):
    for _c in FWD_BLOCKS[_j]:
        FOFF[(_j, _c)] = _off
        _off += P
NFB = _off // P           # 18
# packed col offsets of inverse u (c=0..2, 386 wide) / v (c=3..5, 385)
UOFF = [c * 386 for c in range(3)]
VOFF = [3 * 386 + (c - 3) * 385 for c in (3, 4, 5)]
GW = 3 * 386 + 3 * 385    # 2313


# ---------------- host-side constants (L2-folded DFT) ----------------

def _slot_f_map():
    f = np.zeros(384, dtype=np.int64)
    f[0:128] = 2 * np.arange(128)
    f[128:256] = 2 * np.arange(128) + 1
    f[256:320] = 256 + 2 * np.arange(64)
    f[320:384] = 257 + 2 * np.arange(64)
    return f


def _build_f2g2():
    fmap = _slot_f_map()
    d2 = np.arange(193)
    F2 = np.zeros((768, 768), dtype=np.float64)
    for s in range(384):
        f = fmap[s]
        if f % 2 == 0:
            F2[s, 0:193] = np.cos(2 * np.pi * f * d2 / D)
        else:
            F2[s, 193:385] = np.cos(2 * np.pi * f * np.arange(192) / D)
    F2[384, 0:193] = np.cos(2 * np.pi * 384 * d2 / D)
    for s in range(1, 384):
        f = fmap[s]
        if f % 2 == 0:
            F2[384 + s, 577:768] = -np.sin(
                2 * np.pi * f * np.arange(1, 192) / D)
        else:
            F2[384 + s, 385:577] = -np.sin(
                2 * np.pi * f * np.arange(1, 193) / D)

    G2 = np.zeros((768, 772), dtype=np.float64)
    dd = np.arange(385)
    for s in range(384):
        f = fmap[s]
        w = (1.0 if f == 0 else 2.0) / D
        G2[s, 0:385] = w * np.cos(2 * np.pi * f * dd / D)
    G2[384, 0:385] = (1.0 / D) * np.cos(2 * np.pi * 384 * dd / D)
    dv = np.arange(1, 384)
    for s in range(1, 384):
        f = fmap[s]
        G2[384 + s, 385 + dv] = (2.0 / D) * np.sin(2 * np.pi * f * dv / D)
    return F2, G2


def _build_matrices():
    """Packed nonzero blocks: f2p (128, 16*128), g2p (128, 2699) fp16."""
    F2, G2 = _build_f2g2()
    F2 = F2.copy()
    F2[384, :] = 0.0          # spare row host-injected via z384
    F2T = F2.T
    fb = [F2T[c * P:(c + 1) * P, j * P:(j + 1) * P]
          for j in range(6) for c in FWD_BLOCKS[j]]
    f2p = np.ascontiguousarray(
        np.concatenate(fb, axis=1)).astype(np.float16)
    # u/v blocks with columns REVERSED (pa[:, k] = u_{384-k},
    # pb[:, m] = v_{385-m}) so the PSUM->SBUF uv copy is a plain
    # ascending copy instead of a slower reversed one
    gb = [np.concatenate([G2[c * P:(c + 1) * P, 0:385][:, ::-1],
                          G2[c * P:(c + 1) * P, 385:386]], axis=1)
          for c in range(3)]
    gb += [G2[c * P:(c + 1) * P, 386:771][:, ::-1] for c in (3, 4, 5)]
    g2p = np.ascontiguousarray(
        np.concatenate(gb, axis=1)).astype(np.float16)
    return f2p, g2p


def _build_tables(circ, positions):
    """cos/sin(theta) tables, slot layout, fp16: (2, 128, 1536) each.

    tab[h][p, j*512 + nn] = f(theta(slot=j*128+p, n=h*512+nn)).
    """
    fmap = _slot_f_map()
    S = np.imag(np.fft.rfft(circ.astype(np.float64), axis=-1))  # (2, 385)
    Ss = S[:, fmap]                                             # (2, 384)
    pos = positions.astype(np.float64)                          # (N, 2)
    theta = 2.0 * (pos[:, 0][None, :] * Ss[0][:, None]
                   + pos[:, 1][None, :] * Ss[1][:, None])       # (384, N)

    def to_layout(a):  # (384, N) -> (2, 128, 1536)
        return np.ascontiguousarray(
            a.reshape(3, 128, 2, 512).transpose(2, 1, 0, 3).reshape(
                2, 128, 1536)).astype(np.float16)

    return to_layout(np.cos(theta)), to_layout(np.sin(theta))


def _fold2(x):
    """x (..., 768) fp32 -> eo2 (..., 768)."""
    e = np.zeros(x.shape[:-1] + (385,), dtype=x.dtype)
    e[..., 0] = x[..., 0]
    e[..., 384] = x[..., 384]
    e[..., 1:384] = x[..., 1:384] + x[..., 385:768][..., ::-1]
    o = np.zeros(x.shape[:-1] + (385,), dtype=x.dtype)
    o[..., 1:384] = x[..., 1:384] - x[..., 385:768][..., ::-1]
    eo2 = np.empty_like(x)
    eo2[..., 0] = e[..., 0] + e[..., 384]
    eo2[..., 1:192] = e[..., 1:192] + e[..., 193:384][..., ::-1]
    eo2[..., 192] = e[..., 192]
    eo2[..., 193] = e[..., 0] - e[..., 384]
    eo2[..., 194:385] = e[..., 1:192] - e[..., 193:384][..., ::-1]
    eo2[..., 385:576] = o[..., 1:192] + o[..., 193:384][..., ::-1]
    eo2[..., 576] = o[..., 192]
    eo2[..., 577:768] = o[..., 1:192] - o[..., 193:384][..., ::-1]
    return eo2


# ---------------- device kernel ----------------

def build_kernel(reps=1, trace_sim=False):
    nc = bacc.Bacc("TRN2", target_bir_lowering=False, debug=False,
                   num_devices=NCORES)
    xt = nc.dram_tensor("xt", [BS, 2, P, NCH * ROWTILE], F16,
                        kind="ExternalInput").ap()
    ctab_d = nc.dram_tensor("ctab", [2, P, 1536], F16,
                            kind="ExternalInput").ap()
    stab_d = nc.dram_tensor("stab", [2, P, 1536], F16,
                            kind="ExternalInput").ap()
    f2p_d = nc.dram_tensor("f2p", [P, NFB * P], F16,
                           kind="ExternalInput").ap()
    g2p_d = nc.dram_tensor("g2p", [P, GW], F16, kind="ExternalInput").ap()
    z384_d = nc.dram_tensor("z384", [P, BS * 2 * NG], F16,
                            kind="ExternalInput").ap()
    altc_d = nc.dram_tensor("altc", [P, 385], F16,
                            kind="ExternalInput").ap()
    out16 = nc.dram_tensor("out", [BS, 2, P, NG * D], F16,
                           kind="ExternalOutput").ap()

    with tile.TileContext(nc, trace_sim=trace_sim) as tc, ExitStack() as ctx:
        consts = ctx.enter_context(tc.tile_pool(name="consts", bufs=1))
        tabs = ctx.enter_context(tc.tile_pool(name="tabs", bufs=1))
        xio = ctx.enter_context(tc.tile_pool(name="xio", bufs=2))
        work = ctx.enter_context(tc.tile_pool(name="work", bufs=2))

        # ---- constants on the ACT HWDGE ring, ordered by first use ----
        fpP = consts.tile([P, NFB * P], F16, tag="fpP", name="fpP")
        # j=0 blocks (first two) land first so the first matmul can start
        nc.scalar.dma_start(out=fpP[:, 0:2 * P], in_=f2p_d[:, 0:2 * P])
        nc.scalar.dma_start(out=fpP[:, 2 * P:], in_=f2p_d[:, 2 * P:])
        cTb = [tabs.tile([P, 1536], F16, tag=f"cTb{h}", name=f"cTb{h}")
               for h in range(2)]
        sTb = [tabs.tile([P, 1536], F16, tag=f"sTb{h}", name=f"sTb{h}")
               for h in range(2)]
        nc.scalar.dma_start(out=cTb[0], in_=ctab_d[0])
        nc.scalar.dma_start(out=sTb[0], in_=stab_d[0])
        gpP = consts.tile([P, GW], F16, tag="gpP", name="gpP")
        # later-needed constants go via the gpsimd SWDGE ring; the Pool
        # engine is idle until the first rotation, so these are free and
        # keep the ACT HWDGE ring clear for the first PSUM->SBUF copies.
        nc.gpsimd.dma_start(out=gpP, in_=g2p_d)
        nc.gpsimd.dma_start(out=cTb[1], in_=ctab_d[1])
        nc.gpsimd.dma_start(out=sTb[1], in_=stab_d[1])
        # host-computed f=384 bin, transposed to (n-partition, tile*group)
        # so it can be a per-partition scalar in the inverse, where it
        # enters the folded u via u += z384 * (-1)^k / 768 (one Pool
        # scalar_tensor_tensor per group replaces a 386-col matmul)
        zT = tabs.tile([P, BS * 2 * NG], F16, tag="zT", name="zT")
        nc.gpsimd.dma_start(out=zT, in_=z384_d)
        altc = tabs.tile([P, 385], F16, tag="altc", name="altc")
        nc.gpsimd.dma_start(out=altc, in_=altc_d)

        # ---- main loop (software-pipelined 2 deep: PE runs fwd(k)
        # then inv(k-2), so the rotation of tile k-1 overlaps fwd(k)) ----
        psf = ctx.enter_context(tc.tile_pool(name="psf", bufs=2, space="PSUM"))
        psi = ctx.enter_context(tc.tile_pool(name="psi", bufs=2, space="PSUM"))

        tiles = [(b, h) for _ in range(reps) for b in range(BS)
                 for h in range(2)]
        nt = len(tiles)
        st = {}   # per-tile live state: riR/riI for pending inverse
        lds = {}  # per-tile prefetched xtb

        def issue_load(k, split=False):
            b, h = tiles[k]
            xtb = xio.tile([P, NCH * ROWTILE], F16, tag="xtb", bufs=3)
            if split:
                # chunks {0,1} land first so fwd j=0 can start sooner
                nc.sync.dma_start(out=xtb[:, 0:2 * ROWTILE],
                                  in_=xt[b, h][:, 0:2 * ROWTILE])
                nc.sync.dma_start(out=xtb[:, 2 * ROWTILE:],
                                  in_=xt[b, h][:, 2 * ROWTILE:])
            else:
                nc.sync.dma_start(out=xtb, in_=xt[b, h])
            lds[k] = xtb

        def fwd_j(k, xtb, xRI, j):
            pf = psf.tile([P, 1024], F32, tag="psf", name="pf")
            pR = pf[:, 0:512]
            pI = pf[:, 512:1024]
            kR = FWD_BLOCKS[j]
            for i, c in enumerate(kR):
                o = FOFF[(j, c)]
                nc.tensor.matmul(pR, fpP[:, o:o + P],
                                 xtb[:, c * ROWTILE:(c + 1) * ROWTILE],
                                 start=(i == 0), stop=(i == len(kR) - 1))
            kI = FWD_BLOCKS[3 + j]
            for i, c in enumerate(kI):
                o = FOFF[(3 + j, c)]
                nc.tensor.matmul(pI, fpP[:, o:o + P],
                                 xtb[:, c * ROWTILE:(c + 1) * ROWTILE],
                                 start=(i == 0), stop=(i == len(kI) - 1))
            dst = xRI.rearrange("p (k q) -> p k q",
                                k=2)[:, :, j * 512:(j + 1) * 512]
            src = pf.rearrange("p (k q) -> p k q", k=2)
            nc.scalar.copy(out=dst, in_=src)

        def inv_g(k, riR, riI, osb, g, zci):
            def ri_slice(c):
                if c < 3:
                    return riR[:, c * 512 + g * P: c * 512 + (g + 1) * P]
                return riI[:, (c - 3) * 512 + g * P:
                           (c - 3) * 512 + (g + 1) * P]

            # inverse (folded): u (385) / v (383+pad) in one 2-bank psum
            # tile; merged reversed PSUM->SBUF copy per group into
            # uv = [u_384..u_0 | v-desc], so both un-fold inputs are
            # ascending for the DVE 2x hi-add; gpsimd takes the lo-sub.
            pi_ = psi.tile([P, 1024], F32, tag="pi", name="pi_")
            pa = pi_[:, 0:512]
            pb = pi_[:, 512:1024]
            for i, c in enumerate((0, 1, 2)):
                nc.tensor.matmul(pa[:, 0:386], ri_slice(c),
                                 gpP[:, UOFF[c]:UOFF[c] + 386],
                                 start=(i == 0), stop=(i == 2))
            for i, c in enumerate((3, 4, 5)):
                o = VOFF[c - 3]
                nc.tensor.matmul(pb[:, 0:385], ri_slice(c),
                                 gpP[:, o:o + 385],
                                 start=(i == 0), stop=(i == 2))
            # uv[k]=u_{384-k} (k=0..384), uv[770-d]=v_d (v at
            # cols 386..769 descending; uv[770] memset to 0)
            uv = work.tile([P, 772], F16, tag="uv", bufs=3)
            dst = uv[:, 0:770].rearrange("p (k d) -> p k d", k=2)
            src = pi_.rearrange("p (k d) -> p k d", k=2)[:, :, 0:385]
            nc.scalar.copy(out=dst, in_=src)
            nc.vector.memset(uv[:, 770:772], 0.0)
            # u2[k] = u_{384-k} + z384[n] * (-1)^k / 768: the spare-row
            # (f=384) rank-1 term, with z384 as a per-partition scalar.
            # TensorScalarPtr (AP scalar) is DVE-only on HW.
            u2 = work.tile([P, 385], F16, tag="u2", bufs=3)
            nc.vector.scalar_tensor_tensor(
                u2, altc, zT[:, zci + g:zci + g + 1], uv[:, 0:385],
                op0=mybir.AluOpType.mult, op1=mybir.AluOpType.add)
            gs = g * D
            # lo: out[d] = u_d - v_d (d=0..383; d=0: v_0 slot = 0)
            nc.gpsimd.tensor_sub(osb[:, gs:gs + 384],
                                 u2[:, 384:0:-1], uv[:, 770:386:-1])
            # hi: out[384+m] = u_{384-m} + v_{384-m} (m=0..383)
            nc.vector.tensor_add(osb[:, gs + 384:gs + 768],
                                 u2[:, 0:384], uv[:, 386:770])

        def issue_iter(k):
            """Interleave inv(k-2) groups with fwd(k) j-blocks so the PE
            always has independent work while PSUM slots recycle."""
            front = k < nt
            back = k >= 2
            if front:
                b, h = tiles[k]
                xtb = lds.pop(k)
                xRI = work.tile([P, 3072], F16, tag="xRI", bufs=3)
            if back:
                bb, hb = tiles[k - 2]
                riR, riI = st.pop(k - 2)
                osb = xio.tile([P, NG * D], F16, tag="osb")
                zci = (bb * 2 + hb) * NG
            # PE stream: g0 g1 j0 g2 j1 g3 j2
            if back:
                inv_g(k - 2, riR, riI, osb, 0, zci)
                inv_g(k - 2, riR, riI, osb, 1, zci)
            if front:
                fwd_j(k, xtb, xRI, 0)
            if back:
                inv_g(k - 2, riR, riI, osb, 2, zci)
                nc.sync.dma_start(out=out16[bb, hb][:, 0:2 * D],
                                  in_=osb[:, 0:2 * D])
            if front:
                fwd_j(k, xtb, xRI, 1)
            if back:
                inv_g(k - 2, riR, riI, osb, 3, zci)
                nc.sync.dma_start(out=out16[bb, hb][:, 2 * D:3 * D],
                                  in_=osb[:, 2 * D:3 * D])
            if front:
                fwd_j(k, xtb, xRI, 2)
            if back:
                nc.sync.dma_start(out=out16[bb, hb][:, 3 * D:4 * D],
                                  in_=osb[:, 3 * D:4 * D])
            if front:
                # rotation: 6 fp16 2x tensor ops, split DVE (3) / Pool (3)
                t1 = work.tile([P, 1536], F16, tag="rt1")
                t2 = work.tile([P, 1536], F16, tag="rt2")
                t3 = work.tile([P, 1536], F16, tag="rt3")
                t4 = work.tile([P, 1536], F16, tag="rt4")
                riRn = work.tile([P, 1536], F16, tag="riR", bufs=3)
                riIn = work.tile([P, 1536], F16, tag="riI", bufs=3)
                xRb = xRI[:, 0:1536]
                xIb = xRI[:, 1536:3072]
                nc.vector.tensor_mul(t1, xRb, cTb[h])
                nc.vector.tensor_mul(t2, xIb, sTb[h])
                nc.gpsimd.tensor_mul(t3, xRb, sTb[h])
                nc.gpsimd.tensor_mul(t4, xIb, cTb[h])
                nc.vector.tensor_sub(riRn, t1, t2)
                nc.gpsimd.tensor_add(riIn, t3, t4)
                st[k] = (riRn, riIn)

        issue_load(0, split=True)
        issue_load(1, split=True)
        for k in range(nt + 2):
            if k + 2 < nt:
                issue_load(k + 2)
            issue_iter(k)
    nc.finalize()
    return nc


_NC_CACHE = {}


def _host_prep(x):
    """(BS, N, D) fp32 -> L2-folded (BS, 2, 128, 6*512) fp16 tile layout
    plus the f=384 bin z384 (1, BS*2*512) fp16.

    xtb[p, c*512+r] = eo2[b, h*512+r, c*128+p];
    z384[(b*2+h)*512+r] = sum_d (-1)^d x[b, h*512+r, d].
    """
    eo2 = _fold2(x)
    xt = eo2.reshape(BS, 2, ROWTILE, NCH, P).transpose(0, 1, 4, 3, 2)
    xt = np.ascontiguousarray(xt).astype(np.float16).reshape(
        BS, 2, P, NCH * ROWTILE)
    sgn = np.where(np.arange(D) % 2 == 0, 1.0, -1.0).astype(np.float32)
    z = (x @ sgn).reshape(BS, 2, NG, P)           # [b, h, g, p]
    z384 = np.ascontiguousarray(
        z.transpose(3, 0, 1, 2)).astype(np.float16).reshape(
        P, BS * 2 * NG)                           # [p, (b,h,g)]
    return xt, z384


def _host_post(res16):
    """(BS, 2, 128, 4*768) fp16 -> (BS, N, D) fp32.

    osb[p, g*768+d] = out[b, h*512+g*128+p, d].
    """
    r = res16.reshape(BS, 2, P, NG, D).transpose(0, 1, 3, 2, 4)
    return np.ascontiguousarray(r).astype(np.float32).reshape(BS, N, D)


def make_in_maps(inputs):
    x = np.ascontiguousarray(inputs["x"], dtype=np.float32)
    circ = np.ascontiguousarray(inputs["circ"], dtype=np.float32)
    positions = np.ascontiguousarray(inputs["positions"], dtype=np.int32)
    if "mats" not in _NC_CACHE:
        _NC_CACHE["mats"] = _build_matrices()
    f2p, g2p = _NC_CACHE["mats"]
    ctab, stab = _build_tables(circ, positions)
    altk = np.where(np.arange(385) % 2 == 0, 1.0, -1.0) / D
    altc = np.ascontiguousarray(
        np.broadcast_to(altk, (P, 385))).astype(np.float16)
    in_maps = []
    for core in range(NCORES):
        xt, z384 = _host_prep(x[core * BS:(core + 1) * BS])
        in_maps.append({
            "xt": xt,
            "z384": z384,
            "altc": altc,
            "ctab": ctab,
            "stab": stab,
            "f2p": f2p,
            "g2p": g2p,
        })
    return in_maps


def kernel(x, circ, positions):
    if "nc" not in _NC_CACHE:
        _NC_CACHE["nc"] = build_kernel()
    nc = _NC_CACHE["nc"]
    in_maps = make_in_maps({"x": x, "circ": circ, "positions": positions})
    res = bass_utils.run_bass_kernel_spmd(nc, in_maps,
                                          core_ids=list(range(NCORES)))
    out = np.concatenate(
        [_host_post(res.results[c]["out"]) for c in range(NCORES)], axis=0)
    return out


if __name__ == "__main__":
    rng = np.random.default_rng(0)
    x = rng.standard_normal((B, N, D)).astype(np.float32)
    circ = (rng.standard_normal((2, D)) * 0.01).astype(np.float32)
    positions = rng.integers(0, 32, (N, 2)).astype(np.int32)
    out = kernel(x=x, circ=circ, positions=positions)
    print("out", out.shape, out.dtype)


# revision 45
# speedup vs baseline: 1.3261x; 1.2760x over previous
"""Trainium2 Bass kernel for nn_CirculantSTRING (v7).

Math: out[b,n,:] = irfft(exp(i*theta(n,:)) * rfft(x[b,n,:]), n=D)
where theta(n,f) = 2*(p0[n]*Im(rfft(circ0))[f] + p1[n]*Im(rfft(circ1))[f]).

Sharding: data-parallel over batch, 4 batches per core (8 cores).

Host prep (inside kernel(), per core) — O(input) data prep; all DFT
matmul math runs on device:
  - two-level even/odd fold of x (radix-2 DIF twice) -> eo2 (768 cols)
  - permute to the exact per-tile (partition, chunk*row) layout so each
    tile load is one dense contiguous (128, 3072) fp16 DMA
  - cos/sin phase tables cos(theta)/sin(theta) in slot layout, fp16
  - packed block-sparse constant matrices: f2p = the 16 nonzero
    128x128 blocks of the L2-folded forward DFT (the spare f=384 cos
    row is zeroed out); g2p = the u/v blocks of the folded inverse
    with the u sum trimmed to slot chunks 0-2
  - z384[n] = sum_d (-1)^d x[n,d] (the f=384 bin), shipped transposed
    so it enters the inverse as a per-partition scalar.

Device per (batch, 512-row half):
  - fwd: 16 fp16 matmuls (moving = xtb chunks, N=512) -> PSUM;
    PSUM->SBUF fp16 copies split ACT (j=0,1) / DVE (j=2)
  - rotation: 6 fp16 2x tensor ops split DVE (3) / Pool (3)
  - inverse: 6 fp16 matmuls per 128-row group -> u (386)/v (385) PSUM;
    merged reversed PSUM->SBUF fp16 copy on ACT; the spare-row rank-1
    term added via DVE scalar_tensor_tensor (u2 = altc*z384 + u);
    un-fold split gpsimd (lo-sub) / DVE (hi-add); 3 partial stores to
    a permuted DRAM layout, un-permuted on host.

DMA rings: early constants on the ACT HWDGE ring, later ones on the
gpsimd SWDGE ring, x loads / out stores on the SP ring, so tile
streaming starts at t=0 and the first matmul fires at ~2.5us.
"""
from contextlib import ExitStack

import numpy as np

import concourse.bacc as bacc
import concourse.tile as tile
from concourse import mybir
from concourse import bass_utils

F32 = mybir.dt.float32
F16 = mybir.dt.float16

B, N, D = 32, 1024, 768
NCORES = 8
BS = B // NCORES
P = 128
NCH = D // P              # 6
ROWTILE = 512
NG = ROWTILE // P         # 4

# forward block list (v4 slot/col layout): M-chunk -> list of K-chunks.
# The spare I-slot 384 (f=384 cos row, the only nonzero of chunk 3 in
# d2-chunks 0/1) is host-injected (z384), so j=3 keeps only its sin
# blocks [4, 5].
FWD_BLOCKS = {0: [0, 1], 1: [1, 2, 3], 2: [0, 1, 2, 3],
              3: [4, 5], 4: [3, 4], 5: [3, 4, 5]}
# packed col offset of forward block (j, c) in f2p
FOFF = {}
_off = 0
for _j in range(6):
    for _c in FWD_BLOCKS[_j]:
        FOFF[(_j, _c)] = _off
        _off += P
NFB = _off // P           # 18
# packed col offsets of inverse u (c=0..2, 386 wide) / v (c=3..5, 385)
UOFF = [c * 386 for c in range(3)]
VOFF = [3 * 386 + (c - 3) * 385 for c in (3, 4, 5)]
GW = 3 * 386 + 3 * 385    # 2313


# ---------------- host-side constants (L2-folded DFT) ----------------

def _slot_f_map():
    f = np.zeros(384, dtype=np.int64)
    f[0:128] = 2 * np.arange(128)
    f[128:256] = 2 * np.arange(128) + 1
    f[256:320] = 256 + 2 * np.arange(64)
    f[320:384] = 257 + 2 * np.arange(64)
    return f


def _build_f2g2():
    fmap = _slot_f_map()
    d2 = np.arange(193)
    F2 = np.zeros((768, 768), dtype=np.float64)
    for s in range(384):
        f = fmap[s]
        if f % 2 == 0:
            F2[s, 0:193] = np.cos(2 * np.pi * f * d2 / D)
        else:
            F2[s, 193:385] = np.cos(2 * np.pi * f * np.arange(192) / D)
    F2[384, 0:193] = np.cos(2 * np.pi * 384 * d2 / D)
    for s in range(1, 384):
        f = fmap[s]
        if f % 2 == 0:
            F2[384 + s, 577:768] = -np.sin(
                2 * np.pi * f * np.arange(1, 192) / D)
        else:
            F2[384 + s, 385:577] = -np.sin(
                2 * np.pi * f * np.arange(1, 193) / D)

    G2 = np.zeros((768, 772), dtype=np.float64)
    dd = np.arange(385)
    for s in range(384):
        f = fmap[s]
        w = (1.0 if f == 0 else 2.0) / D
        G2[s, 0:385] = w * np.cos(2 * np.pi * f * dd / D)
    G2[384, 0:385] = (1.0 / D) * np.cos(2 * np.pi * 384 * dd / D)
    dv = np.arange(1, 384)
    for s in range(1, 384):
        f = fmap[s]
        G2[384 + s, 385 + dv] = (2.0 / D) * np.sin(2 * np.pi * f * dv / D)
    return F2, G2


def _build_matrices():
    """Packed nonzero blocks: f2p (128, 16*128), g2p (128, 2699) fp16."""
    F2, G2 = _build_f2g2()
    F2 = F2.copy()
    F2[384, :] = 0.0          # spare row host-injected via z384
    F2T = F2.T
    fb = [F2T[c * P:(c + 1) * P, j * P:(j + 1) * P]
          for j in range(6) for c in FWD_BLOCKS[j]]
    f2p = np.ascontiguousarray(
        np.concatenate(fb, axis=1)).astype(np.float16)
    # u/v blocks with columns REVERSED (pa[:, k] = u_{384-k},
    # pb[:, m] = v_{385-m}) so the PSUM->SBUF uv copy is a plain
    # ascending copy instead of a slower reversed one
    gb = [np.concatenate([G2[c * P:(c + 1) * P, 0:385][:, ::-1],
                          G2[c * P:(c + 1) * P, 385:386]], axis=1)
          for c in range(3)]
    gb += [G2[c * P:(c + 1) * P, 386:771][:, ::-1] for c in (3, 4, 5)]
    g2p = np.ascontiguousarray(
        np.concatenate(gb, axis=1)).astype(np.float16)
    return f2p, g2p


def _build_tables(circ, positions):
    """cos/sin(theta) tables, slot layout, fp16: (2, 128, 1536) each.

    tab[h][p, j*512 + nn] = f(theta(slot=j*128+p, n=h*512+nn)).
    """
    fmap = _slot_f_map()
    S = np.imag(np.fft.rfft(circ.astype(np.float64), axis=-1))  # (2, 385)
    Ss = S[:, fmap]                                             # (2, 384)
    pos = positions.astype(np.float64)                          # (N, 2)
    theta = 2.0 * (pos[:, 0][None, :] * Ss[0][:, None]
                   + pos[:, 1][None, :] * Ss[1][:, None])       # (384, N)

    def to_layout(a):  # (384, N) -> (2, 128, 1536)
        return np.ascontiguousarray(
            a.reshape(3, 128, 2, 512).transpose(2, 1, 0, 3).reshape(
                2, 128, 1536)).astype(np.float16)

    return to_layout(np.cos(theta)), to_layout(np.sin(theta))


def _fold2(x):
    """x (..., 768) fp32 -> eo2 (..., 768)."""
    e = np.zeros(x.shape[:-1] + (385,), dtype=x.dtype)
    e[..., 0] = x[..., 0]
    e[..., 384] = x[..., 384]
    e[..., 1:384] = x[..., 1:384] + x[..., 385:768][..., ::-1]
    o = np.zeros(x.shape[:-1] + (385,), dtype=x.dtype)
    o[..., 1:384] = x[..., 1:384] - x[..., 385:768][..., ::-1]
    eo2 = np.empty_like(x)
    eo2[..., 0] = e[..., 0] + e[..., 384]
    eo2[..., 1:192] = e[..., 1:192] + e[..., 193:384][..., ::-1]
    eo2[..., 192] = e[..., 192]
    eo2[..., 193] = e[..., 0] - e[..., 384]
    eo2[..., 194:385] = e[..., 1:192] - e[..., 193:384][..., ::-1]
    eo2[..., 385:576] = o[..., 1:192] + o[..., 193:384][..., ::-1]
    eo2[..., 576] = o[..., 192]
    eo2[..., 577:768] = o[..., 1:192] - o[..., 193:384][..., ::-1]
    return eo2


# ---------------- device kernel ----------------

def build_kernel(reps=1, trace_sim=False):
    nc = bacc.Bacc("TRN2", target_bir_lowering=False, debug=False,
                   num_devices=NCORES)
    xt = nc.dram_tensor("xt", [BS, 2, P, NCH * ROWTILE], F16,
                        kind="ExternalInput").ap()
    ctab_d = nc.dram_tensor("ctab", [2, P, 1536], F16,
                            kind="ExternalInput").ap()
    stab_d = nc.dram_tensor("stab", [2, P, 1536], F16,
                            kind="ExternalInput").ap()
    f2p_d = nc.dram_tensor("f2p", [P, NFB * P], F16,
                           kind="ExternalInput").ap()
    g2p_d = nc.dram_tensor("g2p", [P, GW], F16, kind="ExternalInput").ap()
    z384_d = nc.dram_tensor("z384", [P, BS * 2 * NG], F16,
                            kind="ExternalInput").ap()
    altc_d = nc.dram_tensor("altc", [P, 385], F16,
                            kind="ExternalInput").ap()
    out16 = nc.dram_tensor("out", [BS, 2, P, NG * D], F16,
                           kind="ExternalOutput").ap()

    with tile.TileContext(nc, trace_sim=trace_sim) as tc, ExitStack() as ctx:
        consts = ctx.enter_context(tc.tile_pool(name="consts", bufs=1))
        tabs = ctx.enter_context(tc.tile_pool(name="tabs", bufs=1))
        xio = ctx.enter_context(tc.tile_pool(name="xio", bufs=2))
        work = ctx.enter_context(tc.tile_pool(name="work", bufs=2))

        # ---- constants on the ACT HWDGE ring, ordered by first use ----
        fpP = consts.tile([P, NFB * P], F16, tag="fpP", name="fpP")
        # j=0 blocks (first two) land first so the first matmul can start
        nc.scalar.dma_start(out=fpP[:, 0:2 * P], in_=f2p_d[:, 0:2 * P])
        nc.scalar.dma_start(out=fpP[:, 2 * P:], in_=f2p_d[:, 2 * P:])
        cTb = [tabs.tile([P, 1536], F16, tag=f"cTb{h}", name=f"cTb{h}")
               for h in range(2)]
        sTb = [tabs.tile([P, 1536], F16, tag=f"sTb{h}", name=f"sTb{h}")
               for h in range(2)]
        nc.scalar.dma_start(out=cTb[0], in_=ctab_d[0])
        nc.scalar.dma_start(out=sTb[0], in_=stab_d[0])
        gpP = consts.tile([P, GW], F16, tag="gpP", name="gpP")
        # later-needed constants go via the gpsimd SWDGE ring; the Pool
        # engine is idle until the first rotation, so these are free and
        # keep the ACT HWDGE ring clear for the first PSUM->SBUF copies.
        nc.gpsimd.dma_start(out=gpP, in_=g2p_d)
        nc.gpsimd.dma_start(out=cTb[1], in_=ctab_d[1])
        nc.gpsimd.dma_start(out=sTb[1], in_=stab_d[1])
        # host-computed f=384 bin, transposed to (n-partition, tile*group)
        # so it can be a per-partition scalar in the inverse, where it
        # enters the folded u via u += z384 * (-1)^k / 768 (one Pool
        # scalar_tensor_tensor per group replaces a 386-col matmul)
        zT = tabs.tile([P, BS * 2 * NG], F16, tag="zT", name="zT")
        nc.gpsimd.dma_start(out=zT, in_=z384_d)
        altc = tabs.tile([P, 385], F16, tag="altc", name="altc")
        nc.gpsimd.dma_start(out=altc, in_=altc_d)

        # ---- main loop (software-pipelined 2 deep: PE runs fwd(k)
        # then inv(k-2), so the rotation of tile k-1 overlaps fwd(k)) ----
        psf = ctx.enter_context(tc.tile_pool(name="psf", bufs=2, space="PSUM"))
        psi = ctx.enter_context(tc.tile_pool(name="psi", bufs=2, space="PSUM"))

        tiles = [(b, h) for _ in range(reps) for b in range(BS)
                 for h in range(2)]
        nt = len(tiles)
        st = {}   # per-tile live state: riR/riI for pending inverse
        lds = {}  # per-tile prefetched xtb

        def issue_load(k, split=False):
            b, h = tiles[k]
            xtb = xio.tile([P, NCH * ROWTILE], F16, tag="xtb", bufs=3)
            if split:
                # chunks {0,1} land first so fwd j=0 can start sooner
                nc.sync.dma_start(out=xtb[:, 0:2 * ROWTILE],
                                  in_=xt[b, h][:, 0:2 * ROWTILE])
                nc.sync.dma_start(out=xtb[:, 2 * ROWTILE:],
                                  in_=xt[b, h][:, 2 * ROWTILE:])
            else:
                nc.sync.dma_start(out=xtb, in_=xt[b, h])
            lds[k] = xtb

        def fwd_j(k, xtb, xRI, j):
            pf = psf.tile([P, 1024], F32, tag="psf", name="pf")
            pR = pf[:, 0:512]
            pI = pf[:, 512:1024]
            kR = FWD_BLOCKS[j]
            for i, c in enumerate(kR):
                o = FOFF[(j, c)]
                nc.tensor.matmul(pR, fpP[:, o:o + P],
                                 xtb[:, c * ROWTILE:(c + 1) * ROWTILE],
                                 start=(i == 0), stop=(i == len(kR) - 1))
            kI = FWD_BLOCKS[3 + j]
            for i, c in enumerate(kI):
                o = FOFF[(3 + j, c)]
                nc.tensor.matmul(pI, fpP[:, o:o + P],
                                 xtb[:, c * ROWTILE:(c + 1) * ROWTILE],
                                 start=(i == 0), stop=(i == len(kI) - 1))
            dst = xRI.rearrange("p (k q) -> p k q",
                                k=2)[:, :, j * 512:(j + 1) * 512]
            src = pf.rearrange("p (k q) -> p k q", k=2)
            nc.scalar.copy(out=dst, in_=src)

        def inv_g(k, riR, riI, osb, g, zci):
            def ri_slice(c):
                if c < 3:
                    return riR[:, c * 512 + g * P: c * 512 + (g + 1) * P]
                return riI[:, (c - 3) * 512 + g * P:
                           (c - 3) * 512 + (g + 1) * P]

            # inverse (folded): u (385) / v (383+pad) in one 2-bank psum
            # tile; merged reversed PSUM->SBUF copy per group into
            # uv = [u_384..u_0 | v-desc], so both un-fold inputs are
            # ascending for the DVE 2x hi-add; gpsimd takes the lo-sub.
            pi_ = psi.tile([P, 1024], F32, tag="pi", name="pi_")
            pa = pi_[:, 0:512]
            pb = pi_[:, 512:1024]
            uv = work.tile([P, 772], F16, tag="uv", bufs=3)
            for i, c in enumerate((0, 1, 2)):
                nc.tensor.matmul(pa[:, 0:386], ri_slice(c),
                                 gpP[:, UOFF[c]:UOFF[c] + 386],
                                 start=(i == 0), stop=(i == 2))
            for i, c in enumerate((3, 4, 5)):
                o = VOFF[c - 3]
                nc.tensor.matmul(pb[:, 0:385], ri_slice(c),
                                 gpP[:, o:o + 385],
                                 start=(i == 0), stop=(i == 2))
            # uv[k]=u_{384-k} (k=0..384), uv[770-d]=v_d (v at
            # cols 386..769 descending; uv[770] memset to 0)
            dst = uv[:, 0:770].rearrange("p (k d) -> p k d", k=2)
            src = pi_.rearrange("p (k d) -> p k d", k=2)[:, :, 0:385]
            nc.scalar.copy(out=dst, in_=src)
            nc.vector.memset(uv[:, 770:772], 0.0)
            # u2[k] = u_{384-k} + z384[n] * (-1)^k / 768: the spare-row
            # (f=384) rank-1 term, with z384 as a per-partition scalar.
            # TensorScalarPtr (AP scalar) is DVE-only on HW.
            u2 = work.tile([P, 385], F16, tag="u2", bufs=3)
            nc.vector.scalar_tensor_tensor(
                u2, altc, zT[:, zci + g:zci + g + 1], uv[:, 0:385],
                op0=mybir.AluOpType.mult, op1=mybir.AluOpType.add)
            gs = g * D
            # lo: out[d] = u_d - v_d (d=0..383; d=0: v_0 slot = 0)
            nc.gpsimd.tensor_sub(osb[:, gs:gs + 384],
                                 u2[:, 384:0:-1], uv[:, 770:386:-1])
            # hi: out[384+m] = u_{384-m} + v_{384-m} (m=0..383)
            nc.vector.tensor_add(osb[:, gs + 384:gs + 768],
                                 u2[:, 0:384], uv[:, 386:770])

        def issue_iter(k):
            """Interleave inv(k-2) groups with fwd(k) j-blocks so the PE
            always has independent work while PSUM slots recycle."""
            front = k < nt
            back = k >= 2
            if front:
                b, h = tiles[k]
                xtb = lds.pop(k)
                xRI = work.tile([P, 3072], F16, tag="xRI", bufs=3)
            if back:
                bb, hb = tiles[k - 2]
                riR, riI = st.pop(k - 2)
                osb = xio.tile([P, NG * D], F16, tag="osb")
                zci = (bb * 2 + hb) * NG
            # PE stream: g0 g1 j0 g2 j1 g3 j2
            if back:
                inv_g(k - 2, riR, riI, osb, 0, zci)
                inv_g(k - 2, riR, riI, osb, 1, zci)
            if front:
                fwd_j(k, xtb, xRI, 0)
            if back:
                inv_g(k - 2, riR, riI, osb, 2, zci)
                nc.sync.dma_start(out=out16[bb, hb][:, 0:2 * D],
                                  in_=osb[:, 0:2 * D])
            if front:
                fwd_j(k, xtb, xRI, 1)
            if back:
                inv_g(k - 2, riR, riI, osb, 3, zci)
                nc.sync.dma_start(out=out16[bb, hb][:, 2 * D:3 * D],
                                  in_=osb[:, 2 * D:3 * D])
            if front:
                fwd_j(k, xtb, xRI, 2)
            if back:
                nc.sync.dma_start(out=out16[bb, hb][:, 3 * D:4 * D],
                                  in_=osb[:, 3 * D:4 * D])
            if front:
                # rotation: 6 fp16 2x tensor ops, split DVE (3) / Pool (3)
                t1 = work.tile([P, 1536], F16, tag="rt1")
                t2 = work.tile([P, 1536], F16, tag="rt2")
                t3 = work.tile([P, 1536], F16, tag="rt3")
                t4 = work.tile([P, 1536], F16, tag="rt4")
                riRn = work.tile([P, 1536], F16, tag="riR", bufs=3)
                riIn = work.tile([P, 1536], F16, tag="riI", bufs=3)
                xRb = xRI[:, 0:1536]
                xIb = xRI[:, 1536:3072]
                nc.vector.tensor_mul(t1, xRb, cTb[h])
                nc.vector.tensor_mul(t2, xIb, sTb[h])
                nc.gpsimd.tensor_mul(t3, xRb, sTb[h])
                nc.gpsimd.tensor_mul(t4, xIb, cTb[h])
                nc.vector.tensor_sub(riRn, t1, t2)
                nc.gpsimd.tensor_add(riIn, t3, t4)
                st[k] = (riRn, riIn)

        issue_load(0, split=True)
        issue_load(1, split=True)
        for k in range(nt + 2):
            if k + 2 < nt:
                issue_load(k + 2)
            issue_iter(k)
    nc.finalize()
    return nc


_NC_CACHE = {}


def _host_prep(x):
    """(BS, N, D) fp32 -> L2-folded (BS, 2, 128, 6*512) fp16 tile layout
    plus the f=384 bin z384 (1, BS*2*512) fp16.

    xtb[p, c*512+r] = eo2[b, h*512+r, c*128+p];
    z384[(b*2+h)*512+r] = sum_d (-1)^d x[b, h*512+r, d].
    """
    eo2 = _fold2(x)
    xt = eo2.reshape(BS, 2, ROWTILE, NCH, P).transpose(0, 1, 4, 3, 2)
    xt = np.ascontiguousarray(xt).astype(np.float16).reshape(
        BS, 2, P, NCH * ROWTILE)
    sgn = np.where(np.arange(D) % 2 == 0, 1.0, -1.0).astype(np.float32)
    z = (x @ sgn).reshape(BS, 2, NG, P)           # [b, h, g, p]
    z384 = np.ascontiguousarray(
        z.transpose(3, 0, 1, 2)).astype(np.float16).reshape(
        P, BS * 2 * NG)                           # [p, (b,h,g)]
    return xt, z384


def _host_post(res16):
    """(BS, 2, 128, 4*768) fp16 -> (BS, N, D) fp32.

    osb[p, g*768+d] = out[b, h*512+g*128+p, d].
    """
    r = res16.reshape(BS, 2, P, NG, D).transpose(0, 1, 3, 2, 4)
    return np.ascontiguousarray(r).astype(np.float32).reshape(BS, N, D)


def make_in_maps(inputs):
    x = np.ascontiguousarray(inputs["x"], dtype=np.float32)
    circ = np.ascontiguousarray(inputs["circ"], dtype=np.float32)
    positions = np.ascontiguousarray(inputs["positions"], dtype=np.int32)
    if "mats" not in _NC_CACHE:
        _NC_CACHE["mats"] = _build_matrices()
    f2p, g2p = _NC_CACHE["mats"]
    ctab, stab = _build_tables(circ, positions)
    altk = np.where(np.arange(385) % 2 == 0, 1.0, -1.0) / D
    altc = np.ascontiguousarray(
        np.broadcast_to(altk, (P, 385))).astype(np.float16)
    in_maps = []
    for core in range(NCORES):
        xt, z384 = _host_prep(x[core * BS:(core + 1) * BS])
        in_maps.append({
            "xt": xt,
            "z384": z384,
            "altc": altc,
            "ctab": ctab,
            "stab": stab,
            "f2p": f2p,
            "g2p": g2p,
        })
    return in_maps


def kernel(x, circ, positions):
    if "nc" not in _NC_CACHE:
        _NC_CACHE["nc"] = build_kernel()
    nc = _NC_CACHE["nc"]
    in_maps = make_in_maps({"x": x, "circ": circ, "positions": positions})
    res = bass_utils.run_bass_kernel_spmd(nc, in_maps,
                                          core_ids=list(range(NCORES)))
    out = np.concatenate(
        [_host_post(res.results[c]["out"]) for c in range(NCORES)], axis=0)
    return out


if __name__ == "__main__":
    rng = np.random.default_rng(0)
    x = rng.standard_normal((B, N, D)).astype(np.float32)
    circ = (rng.standard_normal((2, D)) * 0.01).astype(np.float32)
    positions = rng.integers(0, 32, (N, 2)).astype(np.int32)
    out = kernel(x=x, circ=circ, positions=positions)
    print("out", out.shape, out.dtype)
